# revision 1
# baseline (speedup 1.0000x reference)
"""Trainium2 Bass kernel for nn_Mixture_Loss_74053826118054.

Strategy (pure data parallel: batch axis B=256 sharded over 8 cores):
  Every term of the loss depends only on 5 per-(s,b)-row reductions over D:
    ll = sum_d l^2,  tt = sum_d t^2,  lt = sum_d l*t,
    ln = sum_d l[s]*l[s+1]  (consecutive sentences, same batch),
    tn = sum_d t[s]*t[s+1]
  (masked MSE = sum over valid rows of ll - 2lt + tt; cosines = dots/norms).
  Each core computes those row arrays for its 32 batches; the tiny O(S*B)
  finish (cos, deltas, rank-compaction, delta-of-delta) runs on host in
  float64, reproducing the reference semantics exactly.

Device layout: rows are batch-major (b, s). Each SBUF partition holds a
window of 17 consecutive rows (16 + 1 overlap), so consecutive-row products
are free-axis slices — partition shifts are illegal on compute engines.
l and t are stacked into one DRAM tensor and each 1024-wide chunk (row slot
j of all 128 windows, both halves) is fetched with a single strided DMA.

Per chunk j: ACT does both squares with fused accumulate (Square+accum) and
the lt reduce (Copy+accum); GpSimd computes the lt product; DVE computes the
two shifted products with fused scalar_tensor_tensor+accum. This split
levels the three engines (~85/73/62 us busy); the kernel is DVE-bound at
~91 us vs a ~50 us HBM roofline for the 17.9 MB/core of reads.

The program is hand-scheduled raw bass (no Tile): one engine block each for
Sync (loads + output stores), ACT, DVE and GpSimd, gated by one semaphore
per chunk load (DMA completions are out-of-order across queues) plus a small
ring for the Pool->ACT product handoff. Every chunk gets its own SBUF buffer
(17.4 MB resident) so there are no buffer-recycle waits, and skipping Tile's
entry/exit barriers saves ~4 us.

Toolchain notes: this walrus build encodes at most ONE sync wait per TPB
instruction (standalone wait_ge EventSemaphores everywhere; _legalize_waits
kept as a safety net); tensor_tensor_reduce and the ant custom DVE ops fail
codegen ("ISA wrong length"), and Pool rejects scalar_tensor_tensor, so this
op mix is the fastest legal one. Fused-op full-size `out` operands are
(128,1) dummies broadcast to shape.
"""

import numpy as np

from contextlib import ExitStack

import concourse.bass as bass
import concourse.mybir as mybir
from concourse.bass_utils import run_bass_kernel_spmd

F32 = mybir.dt.float32
AF = mybir.ActivationFunctionType
ALU = mybir.AluOpType

N_CORES = 8
S, B, D = 64, 256, 1024
B_SHARD = B // N_CORES          # 32 batches per core
ROWS = B_SHARD * S              # 2048 real rows per core
G = 16                          # rows per window
P = 128                         # partitions per tile
NMEGA = ROWS // (G * P)         # 2 window-sets per core
ROWS_PAD = (P * NMEGA + 1) * G  # 2056: one extra window of padding rows
NCOL = NMEGA * G                # 16 result columns
QUANTS = ("ll", "tt", "lt", "ln", "tn")
NPC = 6                         # lt-product scratch ring

_cached_nc = None


def _build_program():
    global _cached_nc
    if _cached_nc is not None:
        return _cached_nc
    nc = bass.Bass()
    x_in = nc.dram_tensor("x", [2, ROWS_PAD, D], F32, kind="ExternalInput")
    outs = {q: nc.dram_tensor(q, [P, NCOL], F32, kind="ExternalOutput")
            for q in QUANTS}
    x_v = x_in.rearrange("h (w g) d -> w h g d", g=G)

    with ExitStack() as stack:
        ec = stack.enter_context
        csem = [ec(nc.semaphore(f"c{j}")) for j in range(G + 1)]
        xbig = ec(nc.sbuf_tensor([P, (G + 1) * 2 * D], F32))
        pcs = ec(nc.sbuf_tensor([P, NPC, D], F32))
        dummies = ec(nc.sbuf_tensor([P, 8], F32))
        rll = ec(nc.sbuf_tensor([P, NCOL], F32))
        rtt = ec(nc.sbuf_tensor([P, NCOL], F32))
        rlt = ec(nc.sbuf_tensor([P, NCOL], F32))
        rln = ec(nc.sbuf_tensor([P, NCOL], F32))
        rtn = ec(nc.sbuf_tensor([P, NCOL], F32))
        pool_sem = ec(nc.semaphore("pool_sem"))
        actpc_sem = ec(nc.semaphore("actpc_sem"))
        done_sem = ec(nc.semaphore("done_sem"))
        out_sem = ec(nc.semaphore("out_sem"))
        block = ec(nc.Block())
        rt = {"ll": rll, "tt": rtt, "lt": rlt, "ln": rln, "tn": rtn}
        xc = xbig.ap().rearrange("p (c v d) -> p c v d", v=2, d=D)

        def chunk(j, half):
            return xc[:, j, half, :]

        def bcast(k):
            return dummies.ap()[:, k:k + 1].broadcast_to((P, D))

        @block.sync
        def _(sync):
            for j in range(G + 1):
                src = x_v[0:P, :, j, :] if j < G else x_v[1:P + 1, :, 0, :]
                sync.dma_start(out=xc[:, j, :, :], in_=src).then_inc(
                    csem[j], 16)
            # outputs after all engines finished
            sync.wait_ge(done_sem, 3)
            for q in QUANTS:
                sync.dma_start(out=outs[q][:], in_=rt[q].ap()).then_inc(
                    out_sem, 16)
            sync.wait_ge(out_sem, 16 * len(QUANTS))

        @block.scalar
        def _(scalar):
            for j in range(G):
                scalar.wait_ge(csem[j], 16)
                scalar.activation(bcast(0), chunk(j, 0), AF.Square,
                                  accum_out=rt["ll"].ap()[:, j:j + 1])
                scalar.activation(bcast(1), chunk(j, 1), AF.Square,
                                  accum_out=rt["tt"].ap()[:, j:j + 1])
                scalar.wait_ge(pool_sem, j + 1)
                ins = scalar.activation(
                    bcast(2), pcs.ap()[:, j % NPC, :], AF.Copy,
                    accum_out=rt["lt"].ap()[:, j:j + 1])
                ins.then_inc(actpc_sem, 1)
            scalar.drain().then_inc(done_sem, 1)

        @block.vector
        def _(vector):
            for j in range(G):
                if j == 0:
                    vector.wait_ge(csem[0], 16)
                vector.wait_ge(csem[j + 1], 16)
                vector.scalar_tensor_tensor(
                    out=bcast(3), in0=chunk(j, 0), scalar=0.0,
                    in1=chunk(j + 1, 0), op0=ALU.bypass, op1=ALU.mult,
                    accum_out=rt["ln"].ap()[:, j:j + 1])
                vector.scalar_tensor_tensor(
                    out=bcast(4), in0=chunk(j, 1), scalar=0.0,
                    in1=chunk(j + 1, 1), op0=ALU.bypass, op1=ALU.mult,
                    accum_out=rt["tn"].ap()[:, j:j + 1])
            vector.drain().then_inc(done_sem, 1)

        @block.gpsimd
        def _(gpsimd):
            for j in range(G):
                gpsimd.wait_ge(csem[j], 16)
                if j >= NPC:
                    gpsimd.wait_ge(actpc_sem, j - NPC + 1)
                gpsimd.tensor_tensor(out=pcs.ap()[:, j % NPC, :],
                                     in0=chunk(j, 0), in1=chunk(j, 1),
                                     op=ALU.mult).then_inc(pool_sem, 1)
            gpsimd.drain().then_inc(done_sem, 1)

    _cached_nc = nc
    return nc



def _legalize_waits(nc):
    """Walrus encodes at most one sync wait per TPB instruction. Split any
    non-DMA instruction carrying N>1 waits into N-1 preceding same-engine
    EventSemaphore waits plus the instruction keeping one wait."""
    dummy_sem = nc.alloc_semaphore("legalize_pad")
    cur_insts = nc.cur_bb.bb.instructions
    for bb in nc.main_func.blocks:
        insts = bb.instructions
        new_list = []
        changed = False
        for ins in insts:
            si = ins.sync_info
            waits = list(si.on_wait) if si is not None and si.on_wait else []
            if len(waits) > 1:
                for w in waits[:-1]:
                    ev = nc.engines[ins.engine].wait_ge(dummy_sem, 0).ins
                    # wait_ge appends to the current block; reclaim it
                    popped = cur_insts.pop()
                    assert popped is ev
                    ev.sync_info.on_wait = [w]
                    new_list.append(ev)
                si.on_wait = [waits[-1]]
                changed = True
            new_list.append(ins)
        if changed:
            insts[:] = new_list


def _unpack(arr):
    """(128, NCOL) device layout -> (B_SHARD, S): row r = i*1024 + p*8 + j."""
    return (arr.reshape(P, NMEGA, G).transpose(1, 0, 2)
            .reshape(ROWS).reshape(B_SHARD, S)) if NMEGA > 1 else \
        arr.reshape(ROWS).reshape(B_SHARD, S)


def _run_device(logits, tgt_out, trace=False):
    """Returns dict q -> (B, S) float32 row-dot arrays, plus kernel results."""
    nc = _build_program()
    # (S, B, D) -> (B, S, D) batch-major, split over cores along B
    lb = np.ascontiguousarray(np.swapaxes(logits, 0, 1))
    tb = np.ascontiguousarray(np.swapaxes(tgt_out, 0, 1))
    in_maps = []
    for c in range(N_CORES):
        sl = slice(c * B_SHARD, (c + 1) * B_SHARD)
        x = np.zeros((2, ROWS_PAD, D), np.float32)
        x[0, :ROWS] = lb[sl].reshape(ROWS, D)
        x[1, :ROWS] = tb[sl].reshape(ROWS, D)
        in_maps.append({"x": x})
    kres = run_bass_kernel_spmd(nc, in_maps, list(range(N_CORES)), trace=trace)
    full = {}
    for q in QUANTS:
        full[q] = np.concatenate(
            [_unpack(kres.results[c][q]) for c in range(N_CORES)], axis=0)
    return full, kres


def _finish_host(rows, mask):
    """Host-side float64 finish: reproduce reference semantics exactly."""
    ll = rows["ll"].astype(np.float64)
    tt = rows["tt"].astype(np.float64)
    lt = rows["lt"].astype(np.float64)
    ln = rows["ln"].astype(np.float64)
    tn = rows["tn"].astype(np.float64)

    valid = ~mask                     # (B, S)
    n_valid = float(valid.sum())

    # masked MSE: sum over valid rows of sum_d (l-t)^2 = ll - 2lt + tt
    mse = ((ll - 2.0 * lt + tt) * valid).sum() / (n_valid * D)

    # CosineEmbeddingLoss part (eps = 1e-8)
    na = np.maximum(np.sqrt(ll), 1e-8)
    nb = np.maximum(np.sqrt(tt), 1e-8)
    c = lt / (na * nb)
    loss_cos = ((1.0 - c) * valid).sum() / n_valid

    # consecutive-sentence cosine deltas (eps = 1e-6), shape (B, S-1)
    nl = np.maximum(np.sqrt(ll), 1e-6)
    nt = np.maximum(np.sqrt(tt), 1e-6)
    d_l = ln[:, :S - 1] / (nl[:, :-1] * nl[:, 1:])
    d_t = tn[:, :S - 1] / (nt[:, :-1] * nt[:, 1:])
    pair_valid = valid[:, :-1] & valid[:, 1:]
    cnt = int(pair_valid.sum())
    loss_delta = (np.square(d_l - d_t) * pair_valid).sum() / max(cnt, 1)

    # delta-of-delta on the compacted (valid-only, batch-major) delta lists
    L = B * (S - 1)
    pvf = pair_valid.reshape(-1)

    def dd(d_flat):
        dense = np.zeros(L, np.float64)
        dense[:cnt] = d_flat[pvf]
        prev = dense[:-1]
        den = np.where(prev != 0, prev, 1e-6)
        return (dense[1:] - prev) / den

    dd_l = dd(d_l.reshape(-1))
    dd_t = dd(d_t.reshape(-1))
    dd_valid = np.arange(L - 1) < (cnt - 1)
    n_dd = float(max(cnt - 1, 1))
    loss_dd = (np.square(dd_l - dd_t) * dd_valid).sum() / n_dd / 100.0

    return mse + loss_cos + loss_delta + loss_dd


def kernel(logits, tgt_out, tgt_padding_mask, _trace=False):
    logits = np.asarray(logits, dtype=np.float32)
    tgt_out = np.asarray(tgt_out, dtype=np.float32)
    mask = np.asarray(tgt_padding_mask).astype(bool)
    rows, kres = _run_device(logits, tgt_out, trace=_trace)
    total = _finish_host(rows, mask)
    out = np.array(total, dtype=np.float32)
    if _trace:
        return out, kres
    return out



# revision 2
# speedup vs baseline: 1.1743x; 1.1743x over previous
"""Trainium2 Bass kernel for nn_Mixture_Loss_74053826118054.

Strategy (pure data parallel: batch axis B=256 sharded over 8 cores):
  Every term of the loss depends only on 5 per-(s,b)-row reductions over D:
    ll = sum_d l^2,  tt = sum_d t^2,  lt = sum_d l*t,
    ln = sum_d l[s]*l[s+1]  (consecutive sentences, same batch),
    tn = sum_d t[s]*t[s+1]
  Each core computes those row arrays for its 32 batches; the tiny O(S*B)
  finish (cos, deltas, rank-compaction, delta-of-delta) runs on host in
  float64, reproducing the reference semantics exactly.

Device layout: rows are batch-major (b, s). Each SBUF partition holds a
window of 17 consecutive rows (16 + 1 overlap), so consecutive-row products
are free-axis slices (partition shifts are illegal on compute engines).
l and t are stacked into one DRAM tensor and each 1024-wide chunk (row slot
j of all 128 windows, both halves) is fetched with a single strided DMA.

Engine assignment (v2): profiling the v1 kernel showed DVE's fused
scalar_tensor_tensor runs at ~1213 ns when GpSimd is idle but ~3352 ns
while GpSimd tensor_tensor traffic hits SBUF (2.8x port contention), while
ACT activations are contention-immune at 1131+278 ns. So v2 bans GpSimd:
  ACT: ll, tt squares with fused accumulate        (32 ops, ~45 us busy)
  DVE: ln, tn, lt as fused stt product+accumulate  (48 ops, ~62 us busy)
DMA (17.4 MB/core) streams underneath at ~46 us. No drains: the final
compute op of each engine carries the done increment (sem updates fire
after the read-accumulator aux op per the HW model).
"""

import numpy as np

from contextlib import ExitStack

import concourse.bass as bass
import concourse.mybir as mybir
from concourse.bass_utils import run_bass_kernel_spmd

F32 = mybir.dt.float32
AF = mybir.ActivationFunctionType
ALU = mybir.AluOpType

N_CORES = 8
S, B, D = 64, 256, 1024
B_SHARD = B // N_CORES          # 32 batches per core
ROWS = B_SHARD * S              # 2048 real rows per core
G = 16                          # rows per window
P = 128                         # partitions per tile
NMEGA = ROWS // (G * P)         # 1 window-set per core
ROWS_PAD = (P * NMEGA + 1) * G  # one extra window of padding rows
NCOL = NMEGA * G                # 16 result columns
QUANTS = ("ll", "tt", "lt", "ln", "tn")

_cached_nc = None


def _build_program():
    global _cached_nc
    if _cached_nc is not None:
        return _cached_nc
    nc = bass.Bass()
    x_in = nc.dram_tensor("x", [2, ROWS_PAD, D], F32, kind="ExternalInput")
    res_out = nc.dram_tensor("res", [P, 5 * NCOL], F32, kind="ExternalOutput")
    x_v = x_in.rearrange("h (w g) d -> w h g d", g=G)

    with ExitStack() as stack:
        ec = stack.enter_context
        csem = [ec(nc.semaphore(f"c{j}")) for j in range(G + 1)]
        xbig = ec(nc.sbuf_tensor([P, (G + 1) * 2 * D], F32))
        dummies = ec(nc.sbuf_tensor([P, 8], F32))
        res = ec(nc.sbuf_tensor([P, 5 * NCOL], F32))
        done_sem = ec(nc.semaphore("done_sem"))
        out_sem = ec(nc.semaphore("out_sem"))
        block = ec(nc.Block())
        # result columns: [ll | tt | lt | ln | tn] each NCOL wide
        roff = {q: i * NCOL for i, q in enumerate(QUANTS)}
        xc = xbig.ap().rearrange("p (c v d) -> p c v d", v=2, d=D)

        def chunk(j, half):
            return xc[:, j, half, :]

        def rcol(q, j):
            k = roff[q] + j
            return res.ap()[:, k:k + 1]

        def bcast(k):
            return dummies.ap()[:, k:k + 1].broadcast_to((P, D))

        @block.sync
        def _(sync):
            for j in range(G + 1):
                src = x_v[0:P, :, j, :] if j < G else x_v[1:P + 1, :, 0, :]
                sync.dma_start(out=xc[:, j, :, :], in_=src).then_inc(
                    csem[j], 16)
            # output after both engines finished
            sync.wait_ge(done_sem, 2)
            sync.dma_start(out=res_out[:, :], in_=res.ap()).then_inc(
                out_sem, 16)
            sync.wait_ge(out_sem, 16)

        @block.scalar
        def _(scalar):
            for j in range(G):
                scalar.wait_ge(csem[j], 16)
                ins = scalar.activation(bcast(0), chunk(j, 0), AF.Square,
                                        accum_out=rcol("ll", j))
                ins = scalar.activation(bcast(1), chunk(j, 1), AF.Square,
                                        accum_out=rcol("tt", j))
            ins.then_inc(done_sem, 1)

        @block.vector
        def _(vector):
            for j in range(G):
                if j == 0:
                    vector.wait_ge(csem[0], 16)
                vector.wait_ge(csem[j + 1], 16)
                vector.scalar_tensor_tensor(
                    out=bcast(2), in0=chunk(j, 0), scalar=0.0,
                    in1=chunk(j, 1), op0=ALU.bypass, op1=ALU.mult,
                    accum_out=rcol("lt", j))
                vector.scalar_tensor_tensor(
                    out=bcast(3), in0=chunk(j, 0), scalar=0.0,
                    in1=chunk(j + 1, 0), op0=ALU.bypass, op1=ALU.mult,
                    accum_out=rcol("ln", j))
                ins = vector.scalar_tensor_tensor(
                    out=bcast(4), in0=chunk(j, 1), scalar=0.0,
                    in1=chunk(j + 1, 1), op0=ALU.bypass, op1=ALU.mult,
                    accum_out=rcol("tn", j))
            ins.then_inc(done_sem, 1)

    _cached_nc = nc
    return nc


def _unpack(arr):
    """(128, NCOL) device layout -> (B_SHARD, S): row r = p*G + j."""
    return arr.reshape(ROWS).reshape(B_SHARD, S)


def _run_device(logits, tgt_out, trace=False):
    """Returns dict q -> (B, S) float32 row-dot arrays, plus kernel results."""
    nc = _build_program()
    # (S, B, D) -> (B, S, D) batch-major, split over cores along B
    lb = np.ascontiguousarray(np.swapaxes(logits, 0, 1))
    tb = np.ascontiguousarray(np.swapaxes(tgt_out, 0, 1))
    in_maps = []
    for c in range(N_CORES):
        sl = slice(c * B_SHARD, (c + 1) * B_SHARD)
        x = np.zeros((2, ROWS_PAD, D), np.float32)
        x[0, :ROWS] = lb[sl].reshape(ROWS, D)
        x[1, :ROWS] = tb[sl].reshape(ROWS, D)
        in_maps.append({"x": x})
    kres = run_bass_kernel_spmd(nc, in_maps, list(range(N_CORES)), trace=trace)
    full = {}
    for i, q in enumerate(QUANTS):
        full[q] = np.concatenate(
            [_unpack(kres.results[c]["res"][:, i * NCOL:(i + 1) * NCOL])
             for c in range(N_CORES)], axis=0)
    return full, kres


def _finish_host(rows, mask):
    """Host-side float64 finish: reproduce reference semantics exactly."""
    ll = rows["ll"].astype(np.float64)
    tt = rows["tt"].astype(np.float64)
    lt = rows["lt"].astype(np.float64)
    ln = rows["ln"].astype(np.float64)
    tn = rows["tn"].astype(np.float64)

    valid = ~mask                     # (B, S)
    n_valid = float(valid.sum())

    # masked MSE: sum over valid rows of sum_d (l-t)^2 = ll - 2lt + tt
    mse = ((ll - 2.0 * lt + tt) * valid).sum() / (n_valid * D)

    # CosineEmbeddingLoss part (eps = 1e-8)
    na = np.maximum(np.sqrt(ll), 1e-8)
    nb = np.maximum(np.sqrt(tt), 1e-8)
    c = lt / (na * nb)
    loss_cos = ((1.0 - c) * valid).sum() / n_valid

    # consecutive-sentence cosine deltas (eps = 1e-6), shape (B, S-1)
    nl = np.maximum(np.sqrt(ll), 1e-6)
    nt = np.maximum(np.sqrt(tt), 1e-6)
    d_l = ln[:, :S - 1] / (nl[:, :-1] * nl[:, 1:])
    d_t = tn[:, :S - 1] / (nt[:, :-1] * nt[:, 1:])
    pair_valid = valid[:, :-1] & valid[:, 1:]
    cnt = int(pair_valid.sum())
    loss_delta = (np.square(d_l - d_t) * pair_valid).sum() / max(cnt, 1)

    # delta-of-delta on the compacted (valid-only, batch-major) delta lists
    L = B * (S - 1)
    pvf = pair_valid.reshape(-1)

    def dd(d_flat):
        dense = np.zeros(L, np.float64)
        dense[:cnt] = d_flat[pvf]
        prev = dense[:-1]
        den = np.where(prev != 0, prev, 1e-6)
        return (dense[1:] - prev) / den

    dd_l = dd(d_l.reshape(-1))
    dd_t = dd(d_t.reshape(-1))
    dd_valid = np.arange(L - 1) < (cnt - 1)
    n_dd = float(max(cnt - 1, 1))
    loss_dd = (np.square(dd_l - dd_t) * dd_valid).sum() / n_dd / 100.0

    return mse + loss_cos + loss_delta + loss_dd


def kernel(logits, tgt_out, tgt_padding_mask, _trace=False):
    logits = np.asarray(logits, dtype=np.float32)
    tgt_out = np.asarray(tgt_out, dtype=np.float32)
    mask = np.asarray(tgt_padding_mask).astype(bool)
    rows, kres = _run_device(logits, tgt_out, trace=_trace)
    total = _finish_host(rows, mask)
    out = np.array(total, dtype=np.float32)
    if _trace:
        return out, kres
    return out
